# revision 1
# baseline (speedup 1.0000x reference)
"""2-layer multi-head GAT on 8 Trainium2 NeuronCores.

Sharding: nodes partitioned across 8 cores by dst ownership (6250 nodes each,
padded to 6272 = 49x128). Edges live on their dst's core, sorted by dst into
128-dst blocks. Per layer:
  1. per-core GEMM  feat|el|er = h @ [W | W@Al | W@Ar]   (fp32, PE)
  2. two AllGathers publish every core's projected rows (split in row-halves
     A/B so gather indices fit int16 and the second AG overlaps edge compute)
  3. per dst-block: dma_gather of src rows (1280B) + er rows (256B),
     attention e-chain (DVE/ACT), selection-matrix aggregation matmuls
     accumulated in PSUM (float32r, exact 0/1 lhsT)
  4. flush: divide by softmax denominators, ELU, transpose for next GEMM
"""
import sys
sys.path.insert(0, '/opt/trn_rl_repo')
import numpy as np

N_NODES = 50000
N_EDGES = 800000
IN_DIM = 256
HID = 64
HEADS = 4
NEG_SLOPE = 0.2
N_CORES = 8
NPC = N_NODES // N_CORES          # 6250 real nodes per core
P = 128
NB = 49                            # blocks per core
NPAD = NB * P                      # 6272 padded nodes per core
A_ROWS = 25 * P                    # 3200: local rows in table A
B_ROWS = 24 * P                    # 3072: local rows in table B
RA = N_CORES * A_ROWS              # 25600
RB = N_CORES * B_ROWS              # 24576
ES = 320                           # table row stride/elem (fp32), 1280B
CG = 260                           # feat + el columns
PAD_LDST = 999.0


def _wrap_idx(idx_list):
    """[n] int -> [128, n//16] int16 wrapped-in-16 layout, replicated."""
    n = len(idx_list)
    assert n % 16 == 0
    arr = np.asarray(idx_list, np.int16).reshape(n // 16, 16)  # [s, q]
    w16 = arr.T                                                # [16, s]
    return np.tile(w16, (8, 1))                                # [128, s]


def _prep(x, src, dst, W1, al1, ar1, W2, al2, ar2, kdt=32):
    idt = np.float16 if kdt == 16 else np.float32
    src = np.asarray(src).astype(np.int64)
    dst = np.asarray(dst).astype(np.int64)
    x = np.asarray(x, np.float32)

    # table row id for a global node n
    own = src // NPC
    loc = src % NPC
    in_a = loc < A_ROWS
    rowA = own * A_ROWS + loc                 # valid where in_a
    rowB = own * B_ROWS + (loc - A_ROWS)      # valid where ~in_a

    core_of = (dst // NPC).astype(np.int32)
    ld_all = (dst % NPC).astype(np.int32)
    blk_all = ld_all // P
    lin_all = ld_all % P

    # per (core, block): lists of A-edges and B-edges
    eA = [[[] for _ in range(NB)] for _ in range(N_CORES)]
    eB = [[[] for _ in range(NB)] for _ in range(N_CORES)]
    order = np.lexsort((src, dst))
    for e in order:
        c = core_of[e]
        b = blk_all[e]
        (eA if in_a[e] else eB)[c][b].append(e)

    T_A = [max(1, -(-max(len(eA[c][b]) for c in range(N_CORES)) // P)) for b in range(NB)]
    T_B = [max(1, -(-max(len(eB[c][b]) for c in range(N_CORES)) // P)) for b in range(NB)]
    # allow empty groups
    for b in range(NB):
        if all(len(eA[c][b]) == 0 for c in range(N_CORES)):
            T_A[b] = 0
        if all(len(eB[c][b]) == 0 for c in range(N_CORES)):
            T_B[b] = 0
    T = [T_A[b] + T_B[b] for b in range(NB)]
    TAtot, TBtot, Ttot = sum(T_A), sum(T_B), sum(T)

    plan = {"T_A": T_A, "T_B": T_B}

    # per-core tables
    in_maps = []
    Al1 = np.zeros((IN_DIM, HEADS), np.float64)
    Ar1 = np.zeros((IN_DIM, HEADS), np.float64)
    Al2 = np.zeros((IN_DIM, HEADS), np.float64)
    Ar2 = np.zeros((IN_DIM, HEADS), np.float64)
    for h in range(HEADS):
        Al1[h * HID:(h + 1) * HID, h] = np.asarray(al1, np.float64)[h]
        Ar1[h * HID:(h + 1) * HID, h] = np.asarray(ar1, np.float64)[h]
        Al2[h * HID:(h + 1) * HID, h] = np.asarray(al2, np.float64)[h]
        Ar2[h * HID:(h + 1) * HID, h] = np.asarray(ar2, np.float64)[h]

    def wext(W, Al, Ar):
        W = np.asarray(W, np.float64)
        m = np.concatenate([W, W @ Al, W @ Ar], axis=1)  # [256, 264]
        out = np.zeros((P, 2 * 264), np.float32)
        for g in range(2):
            out[:, g * 264:(g + 1) * 264] = m[g * P:(g + 1) * P].astype(np.float32)
        return out

    W1k = wext(W1, Al1, Ar1)
    W2k = wext(W2, Al2, Ar2)
    iota = np.tile(np.arange(P, dtype=idt), (P, 1))
    ident = np.eye(P, dtype=np.float32)

    for c in range(N_CORES):
        xl = np.zeros((NPAD, IN_DIM), np.float32)
        xl[:NPC] = x[c * NPC:(c + 1) * NPC]
        xT = np.zeros((P, 2 * NPAD), np.float32)
        for g in range(2):
            xT[:, g * NPAD:(g + 1) * NPAD] = xl[:, g * P:(g + 1) * P].T

        idxA_cols = []
        idxB_cols = []
        idxL_cols = []
        ldst_cols = np.full((P, max(Ttot, 1)), PAD_LDST, idt)
        toff = 0
        for b in range(NB):
            ea, eb = eA[c][b], eB[c][b]
            na, nb_ = T_A[b] * P, T_B[b] * P
            ia = [int(rowA[e]) for e in ea] + [0] * (na - len(ea))
            ib = [int(rowB[e]) for e in eb] + [0] * (nb_ - len(eb))
            il = ([int(ld_all[e]) for e in ea] + [0] * (na - len(ea))
                  + [int(ld_all[e]) for e in eb] + [0] * (nb_ - len(eb)))
            lv = ([float(lin_all[e]) for e in ea] + [PAD_LDST] * (na - len(ea))
                  + [float(lin_all[e]) for e in eb] + [PAD_LDST] * (nb_ - len(eb)))
            if na:
                idxA_cols.append(_wrap_idx(ia))
            if nb_:
                idxB_cols.append(_wrap_idx(ib))
            if na + nb_:
                idxL_cols.append(_wrap_idx(il))
                lvm = np.asarray(lv, idt).reshape(T[b], P).T  # [128, T]
                ldst_cols[:, toff:toff + T[b]] = lvm
            toff += T[b]

        in_maps.append({
            "xT": xT,
            "W1k": W1k, "W2k": W2k,
            "idxA": (np.concatenate(idxA_cols, axis=1) if idxA_cols
                     else np.zeros((P, 8), np.int16)),
            "idxB": (np.concatenate(idxB_cols, axis=1) if idxB_cols
                     else np.zeros((P, 8), np.int16)),
            "idxL": (np.concatenate(idxL_cols, axis=1) if idxL_cols
                     else np.zeros((P, 8), np.int16)),
            "ldstT": ldst_cols,
            "iota": iota, "ident": ident,
        })
    plan["idxA_cols"] = in_maps[0]["idxA"].shape[1]
    plan["idxB_cols"] = in_maps[0]["idxB"].shape[1]
    plan["idxL_cols"] = in_maps[0]["idxL"].shape[1]
    plan["ldst_cols"] = in_maps[0]["ldstT"].shape[1]
    plan["kdt"] = kdt
    return in_maps, plan


def _build(plan):
    import os
    KLVL = int(os.environ.get("KLVL", "5"))
    KSIM = int(os.environ.get("KSIM", "0"))
    import concourse.bass as bass
    import concourse.bacc as bacc
    import concourse.mybir as mybir
    import concourse.tile as tile

    dt = mybir.dt
    KDT = plan.get("kdt", 32)
    if KDT == 16:
        FDT = dt.float16          # table/feat dtype
        MDT = dt.float16          # matmul operand dtype for MT/W
        ESL = 384                 # table row elems (768B)
        ELC = 128                 # el col offset in fp32 view of a row
    else:
        FDT = dt.float32
        MDT = dt.float32r
        ESL = ES                  # 320 (1280B)
        ELC = 256
    T_A, T_B = plan["T_A"], plan["T_B"]
    T = [T_A[b] + T_B[b] for b in range(NB)]

    nc = bacc.Bacc("TRN2", target_bir_lowering=False, debug=False,
                   num_devices=(1 if KSIM else N_CORES))
    xT_ap = nc.dram_tensor("xT", [P, 2 * NPAD], dt.float32, kind="ExternalInput").ap()
    W1k_ap = nc.dram_tensor("W1k", [P, 2 * 264], dt.float32, kind="ExternalInput").ap()
    W2k_ap = nc.dram_tensor("W2k", [P, 2 * 264], dt.float32, kind="ExternalInput").ap()
    idxA_ap = nc.dram_tensor("idxA", [P, plan["idxA_cols"]], dt.int16, kind="ExternalInput").ap()
    idxB_ap = nc.dram_tensor("idxB", [P, plan["idxB_cols"]], dt.int16, kind="ExternalInput").ap()
    idxL_ap = nc.dram_tensor("idxL", [P, plan["idxL_cols"]], dt.int16, kind="ExternalInput").ap()
    ldstT_ap = nc.dram_tensor("ldstT", [P, plan["ldst_cols"]], FDT, kind="ExternalInput").ap()
    iota_ap = nc.dram_tensor("iota", [P, P], FDT, kind="ExternalInput").ap()
    ident_ap = nc.dram_tensor("ident", [P, P], dt.float32, kind="ExternalInput").ap()
    out_ap = nc.dram_tensor("out", [NPAD, IN_DIM], dt.float32, kind="ExternalOutput").ap()

    with tile.TileContext(nc) as tc:
        with tc.tile_pool(name="const", bufs=1) as cpool, \
             tc.tile_pool(name="gemm", bufs=3) as gpool, \
             tc.tile_pool(name="edge", bufs=2) as epool, \
             tc.tile_pool(name="flush", bufs=2) as fpool, \
             tc.tile_pool(name="psum", bufs=2, space="PSUM") as pp, \
             tc.tile_pool(name="dram", bufs=1, space="DRAM") as dram:

            iota_t = cpool.tile([P, P], FDT)
            ident_t = cpool.tile([P, P], dt.float32)
            idxA_t = cpool.tile([P, plan["idxA_cols"]], dt.int16)
            idxB_t = cpool.tile([P, plan["idxB_cols"]], dt.int16)
            idxL_t = cpool.tile([P, plan["idxL_cols"]], dt.int16)
            ldst_t = cpool.tile([P, plan["ldst_cols"]], FDT)
            w1_t = cpool.tile([P, 2 * 264], dt.float32)
            w2_t = cpool.tile([P, 2 * 264], dt.float32)
            nc.sync.dma_start(iota_t[:], iota_ap[:])
            nc.sync.dma_start(ident_t[:], ident_ap[:])
            nc.sync.dma_start(idxA_t[:], idxA_ap[:])
            nc.sync.dma_start(idxB_t[:], idxB_ap[:])
            nc.sync.dma_start(idxL_t[:], idxL_ap[:])
            nc.sync.dma_start(ldst_t[:], ldstT_ap[:])
            nc.sync.dma_start(w1_t[:], W1k_ap[:])
            nc.sync.dma_start(w2_t[:], W2k_ap[:])

            tabA_loc = dram.tile([A_ROWS, ESL], FDT)
            tabB_loc = dram.tile([B_ROWS, ESL], FDT)
            _ashared = "Local" if KSIM else "Shared"
            tabA1 = dram.tile([RA, ESL], FDT, addr_space=_ashared)
            tabB1 = dram.tile([RB, ESL], FDT, addr_space=_ashared)
            tabA2 = dram.tile([RA, ESL], FDT, addr_space=_ashared)
            tabB2 = dram.tile([RB, ESL], FDT, addr_space=_ashared)
            er_pad = dram.tile([NPAD, 64], dt.float32)
            h1T = dram.tile([P, 2 * NPAD], dt.float32)

            def gemm_block(layer, b):
                wk = w1_t if layer == 1 else w2_t
                ps = pp.tile([P, 264], dt.float32, space="PSUM", name="gemm_ps", tag="gemm_ps")
                for g in range(2):
                    hk = gpool.tile([P, P], dt.float32, name="hk", tag="hk")
                    if layer == 1:
                        nc.sync.dma_start(hk[:], xT_ap[:, g * NPAD + b * P: g * NPAD + (b + 1) * P])
                    else:
                        nc.sync.dma_start(hk[:], h1T[:, g * NPAD + b * P: g * NPAD + (b + 1) * P])
                    nc.tensor.matmul(out=ps[:], lhsT=hk[:], rhs=wk[:, g * 264:(g + 1) * 264],
                                     start=(g == 0), stop=(g == 1))
                sb = gpool.tile([P, 264], dt.float32, name="gemm_sb", tag="gemm_sb")
                nc.vector.tensor_copy(sb[:], ps[:])
                tab_loc = tabA_loc if b < 25 else tabB_loc
                r0 = b * P if b < 25 else (b - 25) * P
                if KDT == 16:
                    fb16 = gpool.tile([P, 256], dt.float16, name="gemm_f16", tag="gemm_f16")
                    nc.vector.tensor_copy(fb16[:], sb[:, 0:256])
                    nc.sync.dma_start(tab_loc[r0:r0 + P, 0:256], fb16[:])
                    nc.sync.dma_start(
                        tab_loc.bitcast(dt.float32)[r0:r0 + P, ELC:ELC + 4],
                        sb[:, 256:260])
                else:
                    nc.sync.dma_start(tab_loc[r0:r0 + P, 0:CG], sb[:, 0:CG])
                nc.sync.dma_start(er_pad[b * P:(b + 1) * P, 0:4], sb[:, 260:264])

            def edge_block(layer, b, toff, aoff, boff):
                ta, tb_, t = T_A[b], T_B[b], T[b]
                if t == 0 or KLVL < 3:
                    return
                tabA = tabA1 if layer == 1 else tabA2
                tabB = tabB1 if layer == 1 else tabB2
                G = epool.tile([P, t * ESL], FDT, name="G", tag="G")
                if ta:
                    nc.gpsimd.dma_gather(
                        out_ap=G[:, 0:ta * ESL].rearrange("p (t e) -> p t e", e=ESL),
                        in_ap=tabA[:], idxs_ap=idxA_t[:, 8 * aoff: 8 * (aoff + ta)],
                        num_idxs=ta * P, num_idxs_reg=ta * P, elem_size=ESL,
                        single_packet=False)
                if tb_:
                    nc.gpsimd.dma_gather(
                        out_ap=G[:, ta * ESL:t * ESL].rearrange("p (t e) -> p t e", e=ESL),
                        in_ap=tabB[:], idxs_ap=idxB_t[:, 8 * boff: 8 * (boff + tb_)],
                        num_idxs=tb_ * P, num_idxs_reg=tb_ * P, elem_size=ESL,
                        single_packet=False)
                ER = epool.tile([P, t * 64], dt.float32, name="ER", tag="ER")
                nc.gpsimd.dma_gather(
                    out_ap=ER[:].rearrange("p (t e) -> p t e", e=64),
                    in_ap=er_pad[:], idxs_ap=idxL_t[:, 8 * toff: 8 * (toff + t)],
                    num_idxs=t * P, num_idxs_reg=t * P, elem_size=64,
                    single_packet=False)

                if KLVL < 4:
                    return
                gel = G[:].bitcast(dt.float32).rearrange("p (t c) -> p t c", c=ESL // (1 if KDT == 32 else 2))
                er3 = ER[:].rearrange("p (t c) -> p t c", c=64)
                E = epool.tile([P, t * 4], dt.float32, name="E", tag="E")
                e3 = E[:].rearrange("p (t h) -> p t h", h=4)
                nc.vector.tensor_tensor(out=e3, in0=gel[:, :, ELC:ELC + 4],
                                        in1=er3[:, :, 0:4], op=mybir.AluOpType.add)
                L = epool.tile([P, t * 4], dt.float32, name="L", tag="L")
                nc.vector.tensor_scalar_mul(L[:], E[:], NEG_SLOPE)
                nc.vector.tensor_tensor(out=L[:], in0=E[:], in1=L[:],
                                        op=mybir.AluOpType.max)
                X = epool.tile([P, t * 4], dt.float32, name="X", tag="X")
                nc.scalar.activation(X[:], L[:], mybir.ActivationFunctionType.Exp)
                if KDT == 16:
                    XW = epool.tile([P, t * 4], dt.float16, name="XW", tag="XW")
                    nc.vector.tensor_copy(XW[:], X[:])
                else:
                    XW = X

                g3 = G[:].rearrange("p (t c) -> p t c", c=ESL)
                W = epool.tile([P, t * CG], MDT, name="W", tag="W")
                w3 = W[:].rearrange("p (t c) -> p t c", c=CG)
                nc.vector.tensor_copy(w3[:, :, 256:260],
                                      XW[:].rearrange("p (t h) -> p t h", h=4))
                w4 = w3[:, :, 0:256].rearrange("p t (h j) -> p t h j", j=64)
                gf4 = g3[:, :, 0:256].rearrange("p t (h j) -> p t h j", j=64)
                x4 = XW[:].rearrange("p (t h) -> p t h", h=4) \
                          .rearrange("p t (h o) -> p t h o", o=1) \
                          .to_broadcast([P, t, 4, 64])
                nc.vector.tensor_tensor(out=w4, in0=gf4, in1=x4, op=mybir.AluOpType.mult)

                MT = epool.tile([P, t * P], MDT, name="MT", tag="MT")
                mt3 = MT[:].rearrange("p (t c) -> p t c", c=P)
                iota3 = iota_t[:].rearrange("p (o c) -> p o c", o=1).to_broadcast([P, t, P])
                lds3 = ldst_t[:, toff:toff + t].rearrange("p (t o) -> p t o", o=1) \
                                               .to_broadcast([P, t, P])
                nc.vector.tensor_tensor(out=mt3, in0=iota3, in1=lds3,
                                        op=mybir.AluOpType.is_equal)

                agg = pp.tile([P, CG], dt.float32, space="PSUM", name="agg_ps", tag="agg_ps")
                for ti in range(t):
                    nc.tensor.matmul(out=agg[:], lhsT=MT[:, ti * P:(ti + 1) * P],
                                     rhs=W[:, ti * CG:(ti + 1) * CG],
                                     start=(ti == 0), stop=(ti == t - 1))

                # flush: divide by denom, ELU
                dmx = fpool.tile([P, 4], dt.float32, name="dmx", tag="dmx")
                nc.vector.tensor_scalar_max(dmx[:], agg[:, 256:260], 1e-30)
                rec = fpool.tile([P, 4], dt.float32, name="rec", tag="rec")
                nc.vector.reciprocal(rec[:], dmx[:])
                ob = fpool.tile([P, 256], dt.float32, name="ob", tag="ob")
                ob3 = ob[:].rearrange("p (h j) -> p h j", j=64)
                rec3 = rec[:].rearrange("p (h o) -> p h o", o=1).to_broadcast([P, 4, 64])
                nc.vector.tensor_tensor(out=ob3,
                                        in0=agg[:, 0:256].rearrange("p (h j) -> p h j", j=64),
                                        in1=rec3, op=mybir.AluOpType.mult)
                nb_t = fpool.tile([P, 256], dt.float32, name="nb", tag="nb")
                nc.vector.tensor_scalar_min(nb_t[:], ob[:], 0.0)
                en = fpool.tile([P, 256], dt.float32, name="en", tag="en")
                nc.scalar.activation(en[:], nb_t[:], mybir.ActivationFunctionType.Exp)
                pb = fpool.tile([P, 256], dt.float32, name="pb", tag="pb")
                nc.scalar.activation(pb[:], ob[:], mybir.ActivationFunctionType.Relu)
                fb = fpool.tile([P, 256], dt.float32, name="fb", tag="fb")
                nc.vector.tensor_tensor(out=fb[:], in0=en[:], in1=pb[:],
                                        op=mybir.AluOpType.add)
                nc.vector.tensor_scalar_add(fb[:], fb[:], -1.0)

                if KLVL < 5:
                    return
                if layer == 1:
                    for g in range(2):
                        trp = pp.tile([P, P], dt.float32, space="PSUM", name="tr_ps", tag="tr_ps")
                        nc.tensor.transpose(out=trp[:], in_=fb[:, g * P:(g + 1) * P],
                                            identity=ident_t[:])
                        tsb = fpool.tile([P, P], dt.float32, name="tsb", tag="tsb")
                        nc.vector.tensor_copy(tsb[:], trp[:])
                        nc.sync.dma_start(h1T[:, g * NPAD + b * P: g * NPAD + (b + 1) * P], tsb[:])
                else:
                    nc.sync.dma_start(out_ap[b * P:(b + 1) * P, :], fb[:])

            for layer in (1, 2):
                for b in range(25):
                    gemm_block(layer, b)
                if KLVL >= 2 and not KSIM:
                    nc.gpsimd.collective_compute(
                        "AllGather", mybir.AluOpType.bypass,
                        replica_groups=[list(range(N_CORES))],
                        ins=[tabA_loc.opt()],
                        outs=[(tabA1 if layer == 1 else tabA2).opt()])
                for b in range(25, NB):
                    gemm_block(layer, b)
                if KLVL >= 2 and not KSIM:
                    nc.gpsimd.collective_compute(
                        "AllGather", mybir.AluOpType.bypass,
                        replica_groups=[list(range(N_CORES))],
                        ins=[tabB_loc.opt()],
                        outs=[(tabB1 if layer == 1 else tabB2).opt()])
                toff = aoff = boff = 0
                for b in range(NB):
                    edge_block(layer, b, toff, aoff, boff)
                    toff += T[b]
                    aoff += T_A[b]
                    boff += T_B[b]
    nc.compile()
    return nc


def kernel(**inputs):
    import os
    from concourse.bass_utils import run_bass_kernel_spmd
    kdt = int(os.environ.get("KDT", "32"))
    in_maps, plan = _prep(inputs["x"], inputs["src"], inputs["dst"],
                          inputs["W1"], inputs["al1"], inputs["ar1"],
                          inputs["W2"], inputs["al2"], inputs["ar2"], kdt=kdt)
    nc = _build(plan)
    res = run_bass_kernel_spmd(nc, in_maps, core_ids=list(range(N_CORES)),
                               trace=False)
    h = np.concatenate([res.results[c]["out"][:NPC] for c in range(N_CORES)], axis=0)
    return tuple(h[:, i * HID:(i + 1) * HID] for i in range(HEADS))



# revision 4
# speedup vs baseline: 1.5522x; 1.5522x over previous
"""2-layer multi-head GAT on 8 Trainium2 NeuronCores.

Sharding: nodes partitioned across 8 cores by dst ownership (6250 real nodes
per core, padded to 6272 = 49x128). Edges live on their dst's core, sorted by
dst into 128-dst blocks. Per layer:
  1. per-core GEMM  feat|el|er = h @ [W | W@Al | W@Ar]  (fp16 in, fp32 psum)
     -> local table shard [6272, 384] fp16 rows (256 feat + 4 el), er kept
     in a separate local [6272, 128] fp16 table (256B gather rows)
  2. ONE AllGather publishes all shards -> [50176, 384] (core-major order;
     rows < 25088 indexed via table-half A, rest via half B so dma_gather
     int16 indices stay in range)
  3. per 2-block group: dma_gather of src rows (768B) + er rows (256B),
     attention e-chain (DVE/ACT), selection-matrix aggregation matmuls
     accumulated per dst-block in PSUM (fp16 operands, exact 0/1 lhsT)
  4. flush per block: divide by softmax denominators, ELU (fp16), transpose
     for the next GEMM / final output
"""
import sys
sys.path.insert(0, '/opt/trn_rl_repo')
import numpy as np

N_NODES = 50000
N_EDGES = 800000
IN_DIM = 256
HID = 64
HEADS = 4
NEG_SLOPE = 0.2
N_CORES = 8
NPC = N_NODES // N_CORES          # 6250 real nodes per core
P = 128
NB = 49                            # dst blocks per core
NPAD = NB * P                      # 6272 padded nodes per core
HALF = 4 * NPAD                    # 25088: first table half (cores 0-3)
RTOT = N_CORES * NPAD              # 50176 gathered table rows
ES = 384                           # table row elems fp16 (768B)
ERES = 128                         # er table row elems fp16 (256B)
CG = 260                           # feat + denom columns in agg matmul
GB = 2                             # dst blocks per gather group
PAD_LDST = 999.0


def _wrap_idx(idx_list):
    """[n] int -> [128, n//16] int16 wrapped-in-16 layout, replicated."""
    n = len(idx_list)
    assert n % 16 == 0
    arr = np.asarray(idx_list, np.int16).reshape(n // 16, 16)  # [s, q]
    w16 = arr.T                                                # [16, s]
    return np.tile(w16, (8, 1))                                # [128, s]


def _prep(x, src, dst, W1, al1, ar1, W2, al2, ar2, kdt=16):
    src = np.asarray(src).astype(np.int64)
    dst = np.asarray(dst).astype(np.int64)
    x = np.asarray(x, np.float32)

    own = (src // NPC).astype(np.int32)
    loc = (src % NPC).astype(np.int32)
    in_a = own < 4
    rowA = own * NPAD + loc                   # valid where in_a  (< 25088)
    rowB = (own - 4) * NPAD + loc             # valid where ~in_a (< 25088)

    core_of = (dst // NPC).astype(np.int32)
    ld_all = (dst % NPC).astype(np.int32)
    blk_all = ld_all // P
    lin_all = ld_all % P

    # per (core, block): lists of A-edges and B-edges
    eA = [[[] for _ in range(NB)] for _ in range(N_CORES)]
    eB = [[[] for _ in range(NB)] for _ in range(N_CORES)]
    order = np.lexsort((src, dst))
    for e in order:
        c = core_of[e]
        b = blk_all[e]
        (eA if in_a[e] else eB)[c][b].append(e)

    T_A = [max(len(eA[c][b]) for c in range(N_CORES)) for b in range(NB)]
    T_B = [max(len(eB[c][b]) for c in range(N_CORES)) for b in range(NB)]
    T_A = [-(-n // P) for n in T_A]
    T_B = [-(-n // P) for n in T_B]

    # groups of GB consecutive blocks
    groups = [list(range(g, min(g + GB, NB))) for g in range(0, NB, GB)]
    # per group: tile layout [A(b0) A(b1) ... B(b0) B(b1) ...]
    ginfo = []
    for blks in groups:
        ginfo.append({
            "blks": blks,
            "tA": [T_A[b] for b in blks],
            "tB": [T_B[b] for b in blks],
        })
    plan = {"ginfo": ginfo, "T_A": T_A, "T_B": T_B}

    # attention projection: [256, 4] per layer with per-head blocks
    def aext(al, ar):
        Al = np.zeros((IN_DIM, HEADS), np.float64)
        Ar = np.zeros((IN_DIM, HEADS), np.float64)
        for h in range(HEADS):
            Al[h * HID:(h + 1) * HID, h] = np.asarray(al, np.float64)[h]
            Ar[h * HID:(h + 1) * HID, h] = np.asarray(ar, np.float64)[h]
        return Al, Ar

    Al1, Ar1 = aext(al1, ar1)
    Al2, Ar2 = aext(al2, ar2)

    def wext(W, Al, Ar):
        W = np.asarray(W, np.float64)
        m = np.concatenate([W, W @ Al, W @ Ar], axis=1)  # [256, 264]
        out = np.zeros((P, 2 * 264), np.float16)
        for g in range(2):
            out[:, g * 264:(g + 1) * 264] = m[g * P:(g + 1) * P].astype(np.float16)
        return out

    W1k = wext(W1, Al1, Ar1)
    W2k = wext(W2, Al2, Ar2)
    iota = np.tile(np.arange(P, dtype=np.float16), (P, 1))
    ident = np.eye(P, dtype=np.float16)

    in_maps = []
    for c in range(N_CORES):
        xl = np.zeros((NPAD, IN_DIM), np.float32)
        xl[:NPC] = x[c * NPC:(c + 1) * NPC]
        # block-interleaved transpose: xT2[p, b*256 + g*128 + n] = xl[b*128+n, g*128+p]
        xT2 = np.ascontiguousarray(
            xl.reshape(NB, P, 2, P).transpose(3, 0, 2, 1).reshape(P, 2 * NPAD)
        ).astype(np.float16)

        idxA_cols = []
        idxB_cols = []
        idxL_cols = []
        ldst_cols = []
        for gi in ginfo:
            ia, ib, il_a, il_b, lv_a, lv_b = [], [], [], [], [], []
            for k, b in enumerate(gi["blks"]):
                ea, eb = eA[c][b], eB[c][b]
                na, nb_ = gi["tA"][k] * P, gi["tB"][k] * P
                ia += [int(rowA[e]) for e in ea] + [0] * (na - len(ea))
                ib += [int(rowB[e]) for e in eb] + [0] * (nb_ - len(eb))
                il_a += [int(ld_all[e]) for e in ea] + [0] * (na - len(ea))
                il_b += [int(ld_all[e]) for e in eb] + [0] * (nb_ - len(eb))
                lv_a += [float(lin_all[e]) for e in ea] + [PAD_LDST] * (na - len(ea))
                lv_b += [float(lin_all[e]) for e in eb] + [PAD_LDST] * (nb_ - len(eb))
            if ia:
                idxA_cols.append(_wrap_idx(ia))
            if ib:
                idxB_cols.append(_wrap_idx(ib))
            il = il_a + il_b
            lv = lv_a + lv_b
            if il:
                idxL_cols.append(_wrap_idx(il))
                tg = len(lv) // P
                ldst_cols.append(np.asarray(lv, np.float16).reshape(tg, P).T)

        in_maps.append({
            "xT2": xT2,
            "W1k": W1k, "W2k": W2k,
            "idxA": (np.concatenate(idxA_cols, axis=1) if idxA_cols
                     else np.zeros((P, 8), np.int16)),
            "idxB": (np.concatenate(idxB_cols, axis=1) if idxB_cols
                     else np.zeros((P, 8), np.int16)),
            "idxL": (np.concatenate(idxL_cols, axis=1) if idxL_cols
                     else np.zeros((P, 8), np.int16)),
            "ldstT": (np.concatenate(ldst_cols, axis=1) if ldst_cols
                      else np.zeros((P, 1), np.float16)),
            "iota": iota, "ident": ident,
        })
    plan["idxA_cols"] = in_maps[0]["idxA"].shape[1]
    plan["idxB_cols"] = in_maps[0]["idxB"].shape[1]
    plan["idxL_cols"] = in_maps[0]["idxL"].shape[1]
    plan["ldst_cols"] = in_maps[0]["ldstT"].shape[1]
    return in_maps, plan


def _build(plan):
    import os
    KLVL = int(os.environ.get("KLVL", "5"))
    KSIM = int(os.environ.get("KSIM", "0"))
    import concourse.bass as bass
    import concourse.bacc as bacc
    import concourse.mybir as mybir
    import concourse.tile as tile

    dt = mybir.dt
    ginfo = plan["ginfo"]

    nc = bacc.Bacc("TRN2", target_bir_lowering=False, debug=False,
                   num_devices=(1 if KSIM else N_CORES))
    xT2_ap = nc.dram_tensor("xT2", [P, 2 * NPAD], dt.float16, kind="ExternalInput").ap()
    W1k_ap = nc.dram_tensor("W1k", [P, 2 * 264], dt.float16, kind="ExternalInput").ap()
    W2k_ap = nc.dram_tensor("W2k", [P, 2 * 264], dt.float16, kind="ExternalInput").ap()
    idxA_ap = nc.dram_tensor("idxA", [P, plan["idxA_cols"]], dt.int16, kind="ExternalInput").ap()
    idxB_ap = nc.dram_tensor("idxB", [P, plan["idxB_cols"]], dt.int16, kind="ExternalInput").ap()
    idxL_ap = nc.dram_tensor("idxL", [P, plan["idxL_cols"]], dt.int16, kind="ExternalInput").ap()
    ldstT_ap = nc.dram_tensor("ldstT", [P, plan["ldst_cols"]], dt.float16, kind="ExternalInput").ap()
    iota_ap = nc.dram_tensor("iota", [P, P], dt.float16, kind="ExternalInput").ap()
    ident_ap = nc.dram_tensor("ident", [P, P], dt.float16, kind="ExternalInput").ap()
    out_ap = nc.dram_tensor("out", [NPAD, IN_DIM], dt.float16, kind="ExternalOutput").ap()

    with tile.TileContext(nc) as tc:
        with tc.tile_pool(name="const", bufs=1) as cpool, \
             tc.tile_pool(name="gemm", bufs=3) as gpool, \
             tc.tile_pool(name="edge", bufs=2) as epool, \
             tc.tile_pool(name="flush", bufs=2) as fpool, \
             tc.tile_pool(name="psum", bufs=2, space="PSUM") as pp, \
             tc.tile_pool(name="dram", bufs=1, space="DRAM") as dram:

            iota_t = cpool.tile([P, P], dt.float16)
            ident_t = cpool.tile([P, P], dt.float16)
            idxA_t = cpool.tile([P, plan["idxA_cols"]], dt.int16)
            idxB_t = cpool.tile([P, plan["idxB_cols"]], dt.int16)
            idxL_t = cpool.tile([P, plan["idxL_cols"]], dt.int16)
            ldst_t = cpool.tile([P, plan["ldst_cols"]], dt.float16)
            w1_t = cpool.tile([P, 2 * 264], dt.float16)
            w2_t = cpool.tile([P, 2 * 264], dt.float16)
            er_sb = cpool.tile([P, NB * 4], dt.float16)
            nc.sync.dma_start(iota_t[:], iota_ap[:])
            nc.sync.dma_start(ident_t[:], ident_ap[:])
            nc.sync.dma_start(idxA_t[:], idxA_ap[:])
            nc.sync.dma_start(idxB_t[:], idxB_ap[:])
            nc.sync.dma_start(idxL_t[:], idxL_ap[:])
            nc.sync.dma_start(ldst_t[:], ldstT_ap[:])
            nc.sync.dma_start(w1_t[:], W1k_ap[:])
            nc.sync.dma_start(w2_t[:], W2k_ap[:])

            tab_loc = dram.tile([NPAD, ES], dt.float16)
            _ashared = "Local" if KSIM else "Shared"
            tab1 = dram.tile([RTOT, ES], dt.float16, addr_space=_ashared)
            tab2 = dram.tile([RTOT, ES], dt.float16, addr_space=_ashared)
            er_pad = dram.tile([NPAD, ERES], dt.float16)
            h1T2 = dram.tile([P, 2 * NPAD], dt.float16)

            def gemm_block(layer, b):
                wk = w1_t if layer == 1 else w2_t
                hk = gpool.tile([P, 2 * P], dt.float16, name="hk", tag="hk")
                if layer == 1:
                    nc.sync.dma_start(hk[:], xT2_ap[:, b * 256:(b + 1) * 256])
                else:
                    nc.sync.dma_start(hk[:], h1T2[:, b * 256:(b + 1) * 256])
                ps = pp.tile([P, 264], dt.float32, space="PSUM", name="gemm_ps", tag="gemm_ps")
                for g in range(2):
                    nc.tensor.matmul(out=ps[:], lhsT=hk[:, g * P:(g + 1) * P],
                                     rhs=wk[:, g * 264:(g + 1) * 264],
                                     start=(g == 0), stop=(g == 1))
                sb = gpool.tile([P, 264], dt.float16, name="gemm_sb", tag="gemm_sb")
                nc.vector.tensor_copy(sb[:], ps[:])
                nc.sync.dma_start(tab_loc[b * P:(b + 1) * P, 0:CG], sb[:, 0:CG])
                nc.vector.tensor_copy(er_sb[:, b * 4:(b + 1) * 4], sb[:, 260:264])

            def er_flush():
                # er_sb [128, 49*4] -> er_pad rows (b*128+p), cols 0:4
                dst_ap = er_pad[:, 0:4].rearrange("(b p) e -> p b e", p=P)
                nc.sync.dma_start(dst_ap, er_sb[:].rearrange("p (b e) -> p b e", e=4))

            def edge_group(layer, gi, toff, aoff, boff):
                tA, tB = gi["tA"], gi["tB"]
                tgA, tgB = sum(tA), sum(tB)
                tg = tgA + tgB
                if tg == 0 or KLVL < 3:
                    return
                tab = tab1 if layer == 1 else tab2
                G = epool.tile([P, tg * ES], dt.float16, name="G", tag="G")
                if tgA:
                    nc.gpsimd.dma_gather(
                        out_ap=G[:, 0:tgA * ES].rearrange("p (t e) -> p t e", e=ES),
                        in_ap=tab[0:HALF, :], idxs_ap=idxA_t[:, 8 * aoff: 8 * (aoff + tgA)],
                        num_idxs=tgA * P, num_idxs_reg=tgA * P, elem_size=ES,
                        single_packet=False)
                if tgB:
                    nc.gpsimd.dma_gather(
                        out_ap=G[:, tgA * ES:tg * ES].rearrange("p (t e) -> p t e", e=ES),
                        in_ap=tab[HALF:RTOT, :], idxs_ap=idxB_t[:, 8 * boff: 8 * (boff + tgB)],
                        num_idxs=tgB * P, num_idxs_reg=tgB * P, elem_size=ES,
                        single_packet=False)
                ER = epool.tile([P, tg * ERES], dt.float16, name="ER", tag="ER")
                nc.gpsimd.dma_gather(
                    out_ap=ER[:].rearrange("p (t e) -> p t e", e=ERES),
                    in_ap=er_pad[:], idxs_ap=idxL_t[:, 8 * toff: 8 * (toff + tg)],
                    num_idxs=tg * P, num_idxs_reg=tg * P, elem_size=ERES,
                    single_packet=False)

                if KLVL < 4:
                    return
                g3 = G[:].rearrange("p (t c) -> p t c", c=ES)
                er3 = ER[:].rearrange("p (t c) -> p t c", c=ERES)
                E = epool.tile([P, tg * 4], dt.float32, name="E", tag="E")
                e3 = E[:].rearrange("p (t h) -> p t h", h=4)
                nc.vector.tensor_tensor(out=e3, in0=g3[:, :, 256:260],
                                        in1=er3[:, :, 0:4], op=mybir.AluOpType.add)
                L = epool.tile([P, tg * 4], dt.float32, name="L", tag="L")
                nc.vector.tensor_scalar_mul(L[:], E[:], NEG_SLOPE)
                nc.vector.tensor_tensor(out=L[:], in0=E[:], in1=L[:],
                                        op=mybir.AluOpType.max)
                XW = epool.tile([P, tg * 4], dt.float16, name="XW", tag="XW")
                nc.scalar.activation(XW[:], L[:], mybir.ActivationFunctionType.Exp)

                W = epool.tile([P, tg * CG], dt.float16, name="W", tag="W")
                w3 = W[:].rearrange("p (t c) -> p t c", c=CG)
                nc.vector.tensor_copy(w3[:, :, 256:260],
                                      XW[:].rearrange("p (t h) -> p t h", h=4))
                w4 = w3[:, :, 0:256].rearrange("p t (h j) -> p t h j", j=64)
                gf4 = g3[:, :, 0:256].rearrange("p t (h j) -> p t h j", j=64)
                x4 = XW[:].rearrange("p (t h) -> p t h", h=4) \
                          .rearrange("p t (h o) -> p t h o", o=1) \
                          .to_broadcast([P, tg, 4, 64])
                nc.vector.tensor_tensor(out=w4, in0=gf4, in1=x4, op=mybir.AluOpType.mult)

                MT = epool.tile([P, tg * P], dt.float16, name="MT", tag="MT")
                mt3 = MT[:].rearrange("p (t c) -> p t c", c=P)
                iota3 = iota_t[:].rearrange("p (o c) -> p o c", o=1).to_broadcast([P, tg, P])
                lds3 = ldst_t[:, toff:toff + tg].rearrange("p (t o) -> p t o", o=1) \
                                                .to_broadcast([P, tg, P])
                nc.vector.tensor_tensor(out=mt3, in0=iota3, in1=lds3,
                                        op=mybir.AluOpType.is_equal)

                # per-block aggregation + flush
                for k, b in enumerate(gi["blks"]):
                    tiles = (list(range(sum(tA[:k]), sum(tA[:k]) + tA[k]))
                             + list(range(tgA + sum(tB[:k]), tgA + sum(tB[:k]) + tB[k])))
                    if not tiles:
                        continue
                    agg = pp.tile([P, CG], dt.float32, space="PSUM", name="agg_ps", tag="agg_ps")
                    for j, ti in enumerate(tiles):
                        nc.tensor.matmul(out=agg[:], lhsT=MT[:, ti * P:(ti + 1) * P],
                                         rhs=W[:, ti * CG:(ti + 1) * CG],
                                         start=(j == 0), stop=(j == len(tiles) - 1))

                    dmx = fpool.tile([P, 4], dt.float32, name="dmx", tag="dmx")
                    nc.vector.tensor_scalar_max(dmx[:], agg[:, 256:260], 1e-30)
                    rec = fpool.tile([P, 4], dt.float32, name="rec", tag="rec")
                    nc.vector.reciprocal(rec[:], dmx[:])
                    ob = fpool.tile([P, 256], dt.float16, name="ob", tag="ob")
                    ob3 = ob[:].rearrange("p (h j) -> p h j", j=64)
                    rec3 = rec[:].rearrange("p (h o) -> p h o", o=1).to_broadcast([P, 4, 64])
                    nc.vector.tensor_tensor(out=ob3,
                                            in0=agg[:, 0:256].rearrange("p (h j) -> p h j", j=64),
                                            in1=rec3, op=mybir.AluOpType.mult)
                    nb_t = fpool.tile([P, 256], dt.float16, name="nb", tag="nb")
                    nc.vector.tensor_scalar_min(nb_t[:], ob[:], 0.0)
                    en = fpool.tile([P, 256], dt.float16, name="en", tag="en")
                    nc.scalar.activation(en[:], nb_t[:], mybir.ActivationFunctionType.Exp)
                    pb = fpool.tile([P, 256], dt.float16, name="pb", tag="pb")
                    nc.scalar.activation(pb[:], ob[:], mybir.ActivationFunctionType.Relu)
                    fb = fpool.tile([P, 256], dt.float16, name="fb", tag="fb")
                    nc.vector.tensor_tensor(out=fb[:], in0=en[:], in1=pb[:],
                                            op=mybir.AluOpType.add)
                    nc.vector.tensor_scalar_add(fb[:], fb[:], -1.0)

                    if KLVL < 5:
                        continue
                    if layer == 1:
                        tsb = fpool.tile([P, 2 * P], dt.float16, name="tsb", tag="tsb")
                        for g in range(2):
                            trp = pp.tile([P, P], dt.float16, space="PSUM", name="tr_ps", tag="tr_ps")
                            nc.tensor.transpose(out=trp[:], in_=fb[:, g * P:(g + 1) * P],
                                                identity=ident_t[:])
                            nc.vector.tensor_copy(tsb[:, g * P:(g + 1) * P], trp[:])
                        nc.sync.dma_start(h1T2[:, b * 256:(b + 1) * 256], tsb[:])
                    else:
                        nc.sync.dma_start(out_ap[b * P:(b + 1) * P, :], fb[:])

            for layer in (1, 2):
                for b in range(NB):
                    gemm_block(layer, b)
                er_flush()
                if KLVL >= 2 and not KSIM:
                    nc.gpsimd.collective_compute(
                        "AllGather", mybir.AluOpType.bypass,
                        replica_groups=[list(range(N_CORES))],
                        ins=[tab_loc.opt()],
                        outs=[(tab1 if layer == 1 else tab2).opt()])
                toff = aoff = boff = 0
                for gi in ginfo:
                    edge_group(layer, gi, toff, aoff, boff)
                    toff += sum(gi["tA"]) + sum(gi["tB"])
                    aoff += sum(gi["tA"])
                    boff += sum(gi["tB"])
    nc.compile()
    return nc


def kernel(**inputs):
    from concourse.bass_utils import run_bass_kernel_spmd
    in_maps, plan = _prep(inputs["x"], inputs["src"], inputs["dst"],
                          inputs["W1"], inputs["al1"], inputs["ar1"],
                          inputs["W2"], inputs["al2"], inputs["ar2"])
    nc = _build(plan)
    res = run_bass_kernel_spmd(nc, in_maps, core_ids=list(range(N_CORES)),
                               trace=False)
    h = np.concatenate([res.results[c]["out"][:NPC] for c in range(N_CORES)],
                       axis=0).astype(np.float32)
    return tuple(h[:, i * HID:(i + 1) * HID] for i in range(HEADS))
